# revision 34
# baseline (speedup 1.0000x reference)
"""Multi-head attention (B=4, S=2048, D=1024, H=16) on 8 trn2 NeuronCores.

Sharding: core c handles batch c//2 and heads (c%2)*8 .. (c%2)*8+8.
Each core computes its partial output through the fc projection; the host
sums the two per-batch partials.  Keys are compacted on the host (masked
keys dropped, zero-padded to KC*128) which cuts attention work ~44%.

Schedule: one software-pipelined stream over 16 sub-slabs (qb-major:
for each 512-token q-block, all 4 head-pairs), KC kc-steps each:

  step kc: scores^T for both heads of the pair -> one 2-bank PSUM tile
           [128 keys, 2 heads, 512 q] (the two 64-partition matmuls
           co-issue); ONE exp ACT over both banks (per-key -50 bias
           folds the key-padding mask; exp(-50) underflows to exact 0
           in fp16); PV matmuls lag 2 steps so the PE does not wait on
           the ACT engine.

Projection (k/q/v) and fc work interleaves as filler tasks ordered by
data arrival (engine queues are in-order: an instruction waiting on a
late dependency blocks everything behind it).  Loads are
priority-striped over the sync/scalar/gpsimd DMA queues with
chunk-major host layouts; SBUF x-tile layouts are chunk-major too
([128, KC, DC, 128]) so each load piece writes a disjoint byte
interval — Tile's bounding-interval dependency tracking otherwise
serializes consumers on the last-arriving piece.  fc for q-block qb
fills the qb+1 sweep; the final q-block's fc groups pre-accumulate
pairs 0-2 into spare PSUM banks while the last denominator chain flows,
with pair 3 (a dedicated ctxT3 tile) appended after the normalize.

Softmax denominators ride the PV ones-column; per-slab: ACT copies the
two d-rows out of PSUM (partitions 0/32 — engine partition shifts must
be multiples of 32), a repartitioning SBUF->SBUF DMA reshapes them to
[8,128] (DVE reciprocal cost scales with free-dim only), then deferred
one slab later: reciprocal, DRAM bounce, partition-broadcast read, and
one fp16 2x-mode multiply into ctxT.

All matmul operands fp16 (host-cast) with fp32 PSUM accumulation.
"""

import numpy as np

import concourse.bass as bass
import concourse.tile as tile
from concourse import mybir
from concourse.bass_utils import run_bass_kernel_spmd

B, S, DM = 4, 2048, 1024
NH, DEPTH = 16, 64
NCORES = 8
HPC = 8                 # heads per core
C = HPC * DEPTH         # 512 output channels per core
QB = 512                # q block
NQB = S // QB           # 4
DC = DM // 128          # 8 contraction chunks
NPAIR = HPC // 2        # 4 head pairs (= c-tiles of 128)
SCALE = 1.0 / 8.0       # 1/sqrt(depth)
MASK_BIAS = -50.0

F32 = mybir.dt.float32
FP16 = mybir.dt.float16
EXP = mybir.ActivationFunctionType.Exp


def _split_excess_waits(nc, cap_default=1, cap_evsem=2):
    """walrus in this env rejects >1 sync wait per instruction (2 for event
    semaphores); hoist excess waits onto preceding same-engine NoOps."""
    n_split = 0
    for f in nc.m.functions:
        for bb in f.blocks:
            insts = list(bb.instructions)
            out = []
            for inst in insts:
                si = inst.sync_info
                cap = cap_evsem if isinstance(inst, mybir.InstEventSemaphore) else cap_default
                if si is not None and si.on_wait and len(si.on_wait) > cap:
                    waits = list(si.on_wait)
                    extra, keep = waits[:-cap], waits[-cap:]
                    for i, w in enumerate(extra):
                        nop = mybir.InstNoOp(
                            name=f"{inst.name}_waitsplit_{i}",
                            sync_info=mybir.SyncInfo(on_wait=[w], on_update=[]),
                            bass_nofuse=True,
                            engine=inst.engine,
                        )
                        nc.register_instruction(nop, overwrite=True)
                        out.append(nop)
                    inst.sync_info = mybir.SyncInfo(on_wait=keep, on_update=list(si.on_update))
                    n_split += 1
                out.append(inst)
            if n_split:
                bb.instructions = out
    return n_split


def _emit(tc, t, KC):
    SK = KC * 128
    nc = tc.nc
    from contextlib import ExitStack
    ctx = ExitStack()

    persist = ctx.enter_context(tc.tile_pool(name="persist", bufs=1))
    p_a = ctx.enter_context(tc.tile_pool(name="apool", bufs=8))
    p_small = ctx.enter_context(tc.tile_pool(name="small", bufs=3))
    p_d128 = ctx.enter_context(tc.tile_pool(name="d128p", bufs=3))
    p_db = ctx.enter_context(tc.tile_pool(name="dbp", bufs=2))
    p_fcr = ctx.enter_context(tc.tile_pool(name="fcr", bufs=8))
    p_out = ctx.enter_context(tc.tile_pool(name="outsb", bufs=4))
    p_s = ctx.enter_context(tc.tile_pool(name="pss", bufs=2, space="PSUM"))
    p_pv = ctx.enter_context(tc.tile_pool(name="pspv", bufs=2, space="PSUM"))
    p_f = ctx.enter_context(tc.tile_pool(name="psf", bufs=2, space="PSUM"))

    # persistent buffers (x tiles chunk-major: loads hit disjoint byte ranges)
    wq_r = persist.tile([128, DC, C], FP16, tag="wq")
    wk_r = persist.tile([128, DC, C], FP16, tag="wk")
    wv_r = persist.tile([128, DC, C], FP16, tag="wv")
    xq_r = persist.tile([128, NQB, DC, QB], FP16, tag="xq")
    xk_r = persist.tile([128, KC, DC, 128], FP16, tag="xk")
    xv_r = persist.tile([128, KC, DC, 128], FP16, tag="xv")
    qhT = persist.tile([128, NPAIR, S], FP16, tag="qhT")
    khT = persist.tile([128, NPAIR, SK], FP16, tag="khT")
    vhc = persist.tile([128, KC, HPC, DEPTH + 1], FP16, tag="vhc")
    ctxT = persist.tile([128, NPAIR, S], FP16, tag="ctxT")
    ctxT3 = persist.tile([128, QB], FP16, tag="ctxT3")  # last slab only
    maskb = persist.tile([128, KC], F32, tag="maskb")
    ones1 = persist.tile([128, 1], F32, tag="ones1")

    dinv_dram = nc.dram_tensor("dinv_dram", (NPAIR, NQB, 2, 4, 128), FP16,
                               kind="Internal").ap()
    dinv_flat = dinv_dram.rearrange("a b c d e -> (a b c d e)")

    # ---- priority-striped loads; scalar only takes pre-attention pieces ----
    qsrc = t["qT"].rearrange("q p d s -> p q d s")
    ksrc = t["kT"].rearrange("k p d s -> p k d s")
    vsrc = t["vT"].rearrange("k p d s -> p k d s")
    fc_view = t["fcT"].rearrange("(pr p) e -> p pr e", p=128)

    fcrs = {}
    for ec in range(2):
        for pair in range(NPAIR):
            fcrs[(ec, pair)] = p_fcr.tile([128, 512], FP16, tag="fcr",
                                          name=f"fcr_{ec}_{pair}")

    S_, C_, G_ = nc.sync, nc.scalar, nc.gpsimd
    pieces = [
        (S_, maskb[:], t["maskb"]),
        (C_, wk_r[:, 0:4, :], t["wkT"][:, 0:4, :]),
        (G_, wk_r[:, 4:8, :], t["wkT"][:, 4:8, :]),
        (S_, xk_r[:, 0], ksrc[:, 0]),
        (G_, xk_r[:, 1], ksrc[:, 1]),
        (C_, wq_r[:, 0:4, :], t["wqT"][:, 0:4, :]),
        (S_, wq_r[:, 4:8, :], t["wqT"][:, 4:8, :]),
        (G_, xk_r[:, 2], ksrc[:, 2]),
        (C_, xk_r[:, 3], ksrc[:, 3]),
        (G_, xq_r[:, 0, 0:4, :], qsrc[:, 0, 0:4, :]),
        (S_, xq_r[:, 0, 4:8, :], qsrc[:, 0, 4:8, :]),
        (C_, wv_r[:, 0:4, :], t["wvT"][:, 0:4, :]),
        (G_, wv_r[:, 4:8, :], t["wvT"][:, 4:8, :]),
        (S_, xv_r[:, 0], vsrc[:, 0]),
        (C_, xv_r[:, 1], vsrc[:, 1]),
        (G_, xk_r[:, 4], ksrc[:, 4]),
    ]
    eng_alt = [S_, G_]
    ei = 0
    def alt():
        nonlocal ei
        ei += 1
        return eng_alt[ei % 2]
    for kc in range(5, KC):
        pieces.append((alt(), xk_r[:, kc], ksrc[:, kc]))
    for kc in range(2, KC):
        pieces.append((alt(), xv_r[:, kc], vsrc[:, kc]))
    for qb in range(1, NQB):
        for h in range(2):
            d0 = h * 4
            pieces.append((alt(), xq_r[:, qb, d0:d0 + 4, :], qsrc[:, qb, d0:d0 + 4, :]))
    for ec in range(2):
        for pair in range(NPAIR):
            pieces.append((alt(), fcrs[(ec, pair)][:],
                           fc_view[:, pair, ec * 512:(ec + 1) * 512]))
    for eng, dst, src in pieces:
        eng.dma_start(dst, src)

    nc.vector.memset(ones1[:], 1.0)
    nc.vector.tensor_copy(
        vhc[:, :, :, DEPTH:DEPTH + 1],
        ones1[:].to_broadcast([128, KC, HPC, 1]),
    )

    # ---- projection tasks ----
    def k_task(pair, kc0):
        nk = min(2, KC - kc0)
        def task():
            tb0 = kc0 * 128
            ps = p_f.tile([128, 512], F32, tag="f", name=f"kp_{pair}_{kc0}")
            for dc in range(DC):
                nc.tensor.matmul(ps[:, :nk * 128],
                                 wk_r[:, dc, pair * 128:(pair + 1) * 128],
                                 xk_r[:, kc0:kc0 + nk, dc, :],
                                 start=(dc == 0), stop=(dc == DC - 1))
            nc.vector.tensor_copy(khT[:, pair, tb0:tb0 + nk * 128], ps[:, :nk * 128])
        return task

    def k_tasks(pair):
        return [k_task(pair, kc0) for kc0 in range(0, KC, 2)]

    def q_task(pair, qb):
        def task():
            q0 = qb * QB
            ps = p_f.tile([128, 512], F32, tag="f", name=f"qp_{pair}_{qb}")
            for dc in range(DC):
                nc.tensor.matmul(ps[:],
                                 wq_r[:, dc, pair * 128:(pair + 1) * 128],
                                 xq_r[:, qb, dc, :],
                                 start=(dc == 0), stop=(dc == DC - 1))
            nc.vector.tensor_copy(qhT[:, pair, q0:q0 + QB], ps[:])
        return task

    def v_task(kt):
        def task():
            ps = p_f.tile([128, 512], F32, tag="f", name=f"vp_{kt}")
            for dc in range(DC):
                nc.tensor.matmul(ps[:, :C], xv_r[:, kt, dc, :], wv_r[:, dc, :],
                                 start=(dc == 0), stop=(dc == DC - 1))
            nc.vector.tensor_copy(
                vhc[:, kt, :, 0:DEPTH],
                ps[:, :C].rearrange("p (h d) -> p h d", h=HPC),
            )
        return task

    o_view = t["o"].rearrange("(tt p) e -> p tt e", p=128)

    def fc_task(tt, ec, tail=False):
        def task():
            ps = p_f.tile([128, 512], F32, tag="f", name=f"fcps_{tt}_{ec}")
            for pair in range(NPAIR):
                if tail and pair == NPAIR - 1:
                    w = ctxT3[:, (tt - 12) * 128:(tt - 11) * 128]
                else:
                    w = ctxT[:, pair, tt * 128:(tt + 1) * 128]
                nc.tensor.matmul(ps[:], w, fcrs[(ec, pair)][:, :],
                                 start=(pair == 0), stop=(pair == NPAIR - 1))
            ob = p_out.tile([128, 512], F32, tag="outsb", name=f"ob_{tt}_{ec}")
            if tail:
                nc.scalar.copy(ob[:], ps[:])
            else:
                nc.vector.tensor_copy(ob[:], ps[:])
            nc.sync.dma_start(o_view[:, tt, ec * 512:(ec + 1) * 512], ob[:])
        return task

    # ---- one attention sub-slab: (pair, qb) over KC kc-steps, PV lag ----
    # Returns a deferred-finish closure (reciprocal -> DRAM -> broadcast ->
    # normalize) to be emitted a few steps into the NEXT slab.
    def slab(pair, qb, fillers, lag=2, last=False):
        q0 = qb * QB
        ctx_dst = (lambda lo, hi: ctxT3[lo:hi, :]) if last else (
            lambda lo, hi: ctxT[lo:hi, pair, q0:q0 + QB])
        pv = [p_pv.tile([DEPTH + 1, 512], F32, tag="pv",
                        name=f"pv_{pair}_{qb}_{hh}") for hh in range(2)]
        a2s = {}
        steps = KC + lag
        n_fill = len(fillers)
        for st in range(steps):
            if (st < KC and n_fill and
                    st * n_fill // KC != (st + 1) * n_fill // KC):
                for fi in range(st * n_fill // KC, (st + 1) * n_fill // KC):
                    fillers[fi]()
            if st < KC:
                kc = st
                ps2 = p_s.tile([128, 2, 512], F32, tag="s",
                               name=f"s_{pair}_{qb}_{kc}")
                for hh in range(2):
                    lo = 64 * hh
                    nc.tensor.matmul(ps2[:, hh, :],
                                     khT[lo:lo + 64, pair, kc * 128:(kc + 1) * 128],
                                     qhT[lo:lo + 64, pair, q0:q0 + QB],
                                     start=True, stop=True)
                a2 = p_a.tile([128, 2, 512], FP16, tag="A",
                              name=f"A_{pair}_{qb}_{kc}")
                nc.scalar.activation(a2[:], ps2[:], EXP,
                                     bias=maskb[:, kc:kc + 1], scale=SCALE)
                a2s[kc] = a2
            if st >= lag:
                kc = st - lag
                for hh in range(2):
                    nc.tensor.matmul(pv[hh][:], vhc[:, kc, 2 * pair + hh, :],
                                     a2s[kc][:, hh, :],
                                     start=(kc == 0), stop=(kc == KC - 1))

        # drain: ctxT copies (DVE), d-rows out of PSUM (ACT, partitions
        # 0/32), repartition to [8,128] via SBUF->SBUF DMA
        for hh in range(2):
            nc.vector.tensor_copy(ctx_dst(64 * hh, 64 * hh + 64),
                                  pv[hh][0:DEPTH, :])
        dstage = p_small.tile([33, 512], F32, tag="dst", name=f"dst_{pair}_{qb}")
        for hh in range(2):
            nc.scalar.copy(dstage[32 * hh:32 * hh + 1, :],
                           pv[hh][DEPTH:DEPTH + 1, :])
        d128 = p_d128.tile([8, 128], F32, tag="d128", name=f"d128_{pair}_{qb}")
        for hh in range(2):
            nc.gpsimd.dma_start(d128[4 * hh:4 * hh + 4, :],
                                dstage[32 * hh:32 * hh + 1, :])

        def finish():
            db = p_db.tile([128, 512], FP16, tag="db", name=f"db_{pair}_{qb}")
            dinv = p_d128.tile([8, 128], FP16, tag="dinv", name=f"dinv_{pair}_{qb}")
            with nc.allow_low_precision(reason="fp16 1/denom; denom in [1, ~9e3]"):
                nc.vector.reciprocal(dinv[:], d128[:])
            nc.gpsimd.dma_start(dinv_dram[pair, qb], dinv[:])
            for hh in range(2):
                off = ((pair * NQB + qb) * 2 + hh) * 512
                nc.sync.dma_start(db[64 * hh:64 * hh + 64, :],
                                  dinv_flat[off:off + 512].partition_broadcast(64))
            sl = ctx_dst(0, 128)
            nc.vector.tensor_mul(sl, sl, db[:])
        return finish

    # ---- schedule ----
    # lead-in: pair-0 k (first 6 chunks), q(b0), first v chunks; the
    # kc67/kc8 k-tasks land as slab-0 fillers (their loads arrive during
    # the first steps)
    k0 = k_tasks(0)
    for task in k0[:3]:
        task()
    q_task(0, 0)()
    v_task(0)()
    v_task(1)()

    fills = {
        (0, 0): k0[3:] + [v_task(kt) for kt in range(2, KC)] + k_tasks(1) + [q_task(1, 0)],
        (0, 1): k_tasks(2) + [q_task(2, 0)],
        (0, 2): k_tasks(3) + [q_task(3, 0)],
        (0, 3): [q_task(p, 1) for p in range(NPAIR)],
        (1, 0): [q_task(0, 2), q_task(1, 2)],
        (1, 1): [q_task(2, 2), q_task(3, 2)] + [fc_task(0, ec) for ec in range(2)],
        (1, 2): [fc_task(1, ec) for ec in range(2)] + [fc_task(2, 0)],
        (1, 3): [fc_task(2, 1)] + [fc_task(3, ec) for ec in range(2)],
        (2, 0): [q_task(0, 3), q_task(1, 3)],
        (2, 1): [q_task(2, 3), fc_task(4, 0), fc_task(4, 1)],
        (2, 2): [fc_task(5, 0), fc_task(5, 1)],
        (2, 3): [fc_task(6, 0), fc_task(6, 1), fc_task(7, 0)],
        (3, 0): [q_task(3, 3), fc_task(7, 1)],
        (3, 1): [fc_task(8, 0), fc_task(8, 1), fc_task(9, 0)],
        (3, 2): [fc_task(9, 1), fc_task(10, 0), fc_task(10, 1)],
        (3, 3): [fc_task(11, 0), fc_task(11, 1)],
    }
    pending = None
    for qb in range(NQB):
        for pair in range(NPAIR):
            fl = list(fills[(qb, pair)])
            if pending is not None:
                # index 1: one PE filler ahead of the reciprocal so the
                # in-order DVE queue doesn't stall on the d128 DMA chain.
                # Correctness: fc tasks of the just-normalized q-block must
                # come later in the list (see fills layout above).
                fl.insert(min(1, len(fl)), pending)
            pending = slab(pair, qb, fl,
                           lag=4 if (qb == 0 and pair == 0) else 2,
                           last=(qb == NQB - 1 and pair == NPAIR - 1))

    # tail: six qb3-fc groups accumulate pairs 0-2 into six PSUM banks
    # (p_f + p_s halves, both free by now) while the last denominator
    # chain flows; pair-3 matmuls + output copies follow the normalize.
    tail_list = [(12, 0), (12, 1), (13, 0), (13, 1), (14, 0), (14, 1),
                 (15, 0), (15, 1)]
    tail_ps = {}
    ps2t = None
    for i, (tt, ec) in enumerate(tail_list):
        if i < 2:
            ps = p_f.tile([128, 512], F32, tag="f", name=f"tf_{tt}_{ec}")
        elif i < 6:
            if i % 2 == 0:
                ps2t = p_s.tile([128, 2, 512], F32, tag="s", name=f"tf2_{tt}_{ec}")
            ps = ps2t[:, i % 2, :]
        else:
            ps = p_pv.tile([128, 512], F32, tag="pv", name=f"tfv_{tt}_{ec}")
        tail_ps[(tt, ec)] = ps
        for pair in range(NPAIR - 1):
            nc.tensor.matmul(ps[:], ctxT[:, pair, tt * 128:(tt + 1) * 128],
                             fcrs[(ec, pair)][:, :],
                             start=(pair == 0), stop=False)
    pending()
    for tt, ec in tail_list:
        ps = tail_ps[(tt, ec)]
        nc.tensor.matmul(ps[:], ctxT3[:, (tt - 12) * 128:(tt - 11) * 128],
                         fcrs[(ec, NPAIR - 1)][:, :], start=False, stop=True)
        ob = p_out.tile([128, 512], F32, tag="outsb", name=f"tob_{tt}_{ec}")
        nc.scalar.copy(ob[:], ps[:])
        nc.sync.dma_start(o_view[:, tt, ec * 512:(ec + 1) * 512], ob[:])

    ctx.close()


_NC_CACHE = {}


def _get_nc(KC):
    if KC in _NC_CACHE:
        return _NC_CACHE[KC]
    nc = bass.Bass("TRN2", target_bir_lowering=False, debug=False)
    t = {
        "qT": nc.dram_tensor("qT", (NQB, 128, DC, QB), FP16, kind="ExternalInput").ap(),
        "kT": nc.dram_tensor("kT", (KC, 128, DC, 128), FP16, kind="ExternalInput").ap(),
        "vT": nc.dram_tensor("vT", (KC, 128, DC, 128), FP16, kind="ExternalInput").ap(),
        "wqT": nc.dram_tensor("wqT", (128, DC, C), FP16, kind="ExternalInput").ap(),
        "wkT": nc.dram_tensor("wkT", (128, DC, C), FP16, kind="ExternalInput").ap(),
        "wvT": nc.dram_tensor("wvT", (128, DC, C), FP16, kind="ExternalInput").ap(),
        "fcT": nc.dram_tensor("fcT", (C, DM), FP16, kind="ExternalInput").ap(),
        "maskb": nc.dram_tensor("maskb", (128, KC), F32, kind="ExternalInput").ap(),
        "o": nc.dram_tensor("o", (S, DM), F32, kind="ExternalOutput").ap(),
    }
    with tile.TileContext(nc) as tc:
        _emit(tc, t, KC)
    _split_excess_waits(nc)
    _NC_CACHE[KC] = nc
    return nc


def _to_chunk_major(x2d, nchunk, chunk):
    """[nchunk*chunk, DM] -> [nchunk, 128, DC, chunk] fp16 (partition-contig)."""
    return np.ascontiguousarray(
        x2d.reshape(nchunk, chunk, DC, 128).transpose(0, 3, 2, 1).astype(np.float16))


def _w_chunk_major(w):
    """[C-rows, DM] weight slice -> [128, DC, C] fp16 (w.T, partition-major)."""
    return np.ascontiguousarray(
        w.T.reshape(DC, 128, C).transpose(1, 0, 2).astype(np.float16))


def _in_map_for_core(core, KC, v, k, q, mask, wq, wk, wv, fc):
    SK = KC * 128
    b = core // 2
    hs = (core % 2) * HPC
    sel = np.nonzero(mask[b] == 0)[0]
    n = len(sel)
    assert n <= SK, f"unmasked key count {n} exceeds SK={SK}"
    kc_ = np.zeros((SK, DM), np.float32)
    kc_[:n] = k[b][sel]
    vc_ = np.zeros((SK, DM), np.float32)
    vc_[:n] = v[b][sel]
    mb = np.full(SK, MASK_BIAS, np.float32)
    mb[:n] = 0.0
    return {
        "qT": _to_chunk_major(q[b], NQB, QB),
        "kT": _to_chunk_major(kc_, KC, 128),
        "vT": _to_chunk_major(vc_, KC, 128),
        "wqT": _w_chunk_major(wq[hs * DEPTH:(hs + HPC) * DEPTH]),
        "wkT": _w_chunk_major(wk[hs * DEPTH:(hs + HPC) * DEPTH]),
        "wvT": _w_chunk_major(wv[hs * DEPTH:(hs + HPC) * DEPTH]),
        "fcT": np.ascontiguousarray(fc[:, hs * DEPTH:(hs + HPC) * DEPTH].T.astype(np.float16)),
        "maskb": np.ascontiguousarray(mb.reshape(KC, 128).T),
    }


def kernel(v, k, q, mask, wq, wk, wv, fc, _run_kwargs=None):
    v = np.asarray(v, np.float32)
    k = np.asarray(k, np.float32)
    q = np.asarray(q, np.float32)
    mask = np.asarray(mask)
    wq = np.asarray(wq, np.float32)
    wk = np.asarray(wk, np.float32)
    wv = np.asarray(wv, np.float32)
    fc = np.asarray(fc, np.float32)

    max_n = max(int((mask[b] == 0).sum()) for b in range(B))
    KC = max(5, -(-max_n // 128))
    nc = _get_nc(KC)
    in_maps = [_in_map_for_core(c, KC, v, k, q, mask, wq, wk, wv, fc)
               for c in range(NCORES)]
    res = run_bass_kernel_spmd(nc, in_maps, core_ids=list(range(NCORES)),
                               **(_run_kwargs or {}))
    outs = [r["o"] for r in res.results]
    full = np.stack([outs[2 * b] + outs[2 * b + 1] for b in range(B)])
    if _run_kwargs:
        kernel.last_results = res
    return full
